# revision 9
# baseline (speedup 1.0000x reference)
"""Trainium2 Bass kernel for DropChannel (topk channel masking).

Math (per sample):
    score_c = mean_hw x[hw, c]                       (only sums needed; 1/HW cancels)
    lk_c    = ln(r_c) * (1 / S_c)                    (log of key r**(1/score); order-preserving)
    gcnt_i  = #{c : lk_c > lk_i}                     (strictly-greater count)
    sel_i   = gcnt_i < C - M                         (identical to thr = sort(key)[C-M]; sel = key >= thr,
                                                      including tie behaviour)
    alpha   = sum(S) / sum(S * sel)
    out     = x * (sel & (u < P)) * alpha

Sharding: pure data parallel, N=32 samples -> 8 cores x 4 samples.

Per-core schedule (4 samples, each [4096, 1024] f32), built so the HBM
stream never idles:
  - x tiles are [128, 2048] (two hw rows per partition line -> 8 KiB
    descriptors; 4 KiB descriptors were read-latency bound)
  - tiles land in a small f32 staging pool; each pair is summed in place
    on the DVE and fp32 score matmuls (needed for bit-exact top-k
    selection) run per pair -- halving fp32 PE work keeps stage recycle
    ahead of the load stream
  - every engine class is load-balanced so no single engine gates the
    pipeline: the f32->bf16 resident casts alternate DVE / ACT, the
    pass-2 multiplies alternate DVE / GPSIMD, and the gcnt compares
    split DVE / GPSIMD -- so sample s+1 keeps loading at full HBM rate
    while sample s runs its mask chain and store burst
  - bf16 residency halves SBUF: TWO full samples fit resident
  - the mask chain uses NO DMAs: row->column and column->row reshapes of
    the tiny score/key vectors run on the PE (K=1 column matmuls,
    per-chunk identity transposes, rank-1 broadcast matmuls); tiny
    SBUF-SBUF DMAs took ~6us each behind 1 MiB load packets
  - pass 2 multiplies the bf16 tiles in place (mask 0/1 bf16 exact,
    alpha an f32 per-partition scalar) and stores via SWDGE DMA with an
    inline bf16->f32 cast (output precision ~3e-3 relative, gate 2e-2)
  - queue separation: x loads on qSP (sync), r/u loads on qACT (scalar),
    stores on SWDGE (gpsimd)
"""

import numpy as np
from contextlib import ExitStack

import concourse.bacc as bacc
import concourse.tile as tile
from concourse import masks, mybir
from concourse.bass_utils import run_bass_kernel_spmd

N, HW, C = 32, 4096, 1024
NCORES = 8
NS = N // NCORES          # samples per core
P = 128                   # partitions
CK = C // P               # 8 channel chunks; channel c = k*128 + p
NKEEP = C - int(0.5 * C)  # gcnt threshold: keep rows with gcnt < 512
PKEEP = 0.9
HALF = 512                # matmul free-dim limit (one PSUM bank)
TWO = 2                   # hw rows per partition line
FREE = TWO * C            # 2048 free dim per tile
NT = HW // (P * TWO)      # 16 tiles per sample

f32 = mybir.dt.float32
bf16 = mybir.dt.bfloat16
ALU = mybir.AluOpType
ACTF = mybir.ActivationFunctionType
AXIS = mybir.AxisListType

# The score sums must stay fp32: the selection boundary sits only ~2e-5
# (relative) away from the threshold, so scores are matmul'd from the f32
# staging tiles before the bf16 cast. The 0/1 comparison tiles are bf16
# (exact for 0/1, full-rate PE matmul).


def emit(tc, o, x, r, u, ns, hw):
    nc = tc.nc
    nt = hw // (P * TWO)
    xt = x.rearrange("s (t p two) c -> s t p (two c)", p=P, two=TWO)
    ot = o.rearrange("s (t p two) c -> s t p (two c)", p=P, two=TWO)
    # column layout: partition p of chunk k holds channel k*128 + p, so a
    # 128-wide slice of a row vector PE-transposes straight into a column
    rck = r.rearrange("s (k p) -> s p k", p=P)

    with ExitStack() as ctx:
        stage = ctx.enter_context(tc.tile_pool(name="stage", bufs=4))
        xpool = ctx.enter_context(tc.tile_pool(name="xpool", bufs=2 * NT))
        tqpool = ctx.enter_context(tc.tile_pool(name="tqpool", bufs=4))
        bcpool = ctx.enter_context(tc.tile_pool(name="bcpool", bufs=2))
        rows = ctx.enter_context(tc.tile_pool(name="rows", bufs=1))
        consts = ctx.enter_context(tc.tile_pool(name="consts", bufs=1))
        ps_s = ctx.enter_context(tc.tile_pool(name="ps_s", bufs=2, space="PSUM"))
        ps_mid = ctx.enter_context(tc.tile_pool(name="ps_mid", bufs=1, space="PSUM"))

        ones_col = consts.tile([P, 1], f32)
        nc.vector.memset(ones_col, 1.0)
        ones_b = consts.tile([P, 1], bf16)
        nc.vector.memset(ones_b, 1.0)
        ones_11 = consts.tile([1, 1], f32)
        nc.vector.memset(ones_11, 1.0)
        ones_row = consts.tile([1, P], f32)
        nc.vector.memset(ones_row, 1.0)
        ident = consts.tile([P, P], f32)
        masks.make_identity(nc, ident)

        for s in range(ns):
            # precompute pieces that do not depend on x (overlap with loads):
            # ln(r) in (p k) layout, and the bernoulli gate row (u < PKEEP)
            lnr_cols = rows.tile([P, CK], f32, tag="lnr_cols", bufs=2)
            nc.scalar.dma_start(out=lnr_cols, in_=rck[s])
            nc.scalar.activation(lnr_cols, lnr_cols, ACTF.Ln)
            rng_row = rows.tile([1, C], f32, tag="rng_row", bufs=2)
            nc.scalar.dma_start(out=rng_row, in_=u[s:s + 1, :])
            nc.vector.tensor_scalar(rng_row, rng_row, PKEEP, None, op0=ALU.is_lt)

            # ---- pass 1: stage f32 tile pairs, pair-add + score matmuls,
            # cast each tile to a bf16 resident copy ----
            ps_score = ps_s.tile([1, C], f32, tag="ps_score")
            xts = []
            for t in range(nt):
                xstage = stage.tile([P, FREE], f32, tag="xstage")
                nc.sync.dma_start(out=xstage, in_=xt[s, t])
                xres = xpool.tile([P, FREE], bf16, tag="xres")
                if t % 2 == 0:
                    nc.vector.tensor_copy(xres, xstage)
                else:
                    nc.scalar.copy(xres, xstage)
                xts.append(xres)
                if t % 2 == 0:
                    pair_lo = xstage
                else:
                    # in-place pair sum (WAR on pair_lo's cast is tracked by
                    # Tile); 4 fp32 matmuls per pair accumulate the scores
                    nc.vector.tensor_add(pair_lo, pair_lo, xstage)
                    for g in range(TWO):
                        for h in range(2):
                            nc.tensor.matmul(
                                ps_score[:, h * HALF:(h + 1) * HALF],
                                lhsT=ones_col,
                                rhs=pair_lo[:, g * C + h * HALF:g * C + (h + 1) * HALF],
                                start=(t == 1 and g == 0),
                                stop=(t == nt - 1 and g == TWO - 1),
                            )

            # ---- mid: selection mask (DMA-free, PE does the reshapes) ----
            s_row = rows.tile([1, C], f32, tag="s_row")
            nc.scalar.copy(s_row[:, 0:HALF], ps_score[:, 0:HALF])
            nc.vector.tensor_copy(s_row[:, HALF:], ps_score[:, HALF:])
            # row -> column: K=1 matmuls drop each 128-wide row chunk into a
            # PSUM column; reciprocal + logkey in column layout (128-way).
            # scols / lkrow / gcnt share one PSUM slot (disjoint lifetimes):
            # 8 banks = score 2x2 + scratch 2 + bbc 2
            ps_scols = ps_mid.tile([P, CK], f32, tag="ps_scratch")
            for k in range(CK):
                nc.tensor.matmul(
                    ps_scols[:, k:k + 1],
                    lhsT=s_row[:, k * P:(k + 1) * P],
                    rhs=ones_11,
                    start=True, stop=True,
                )
            recip_cols = rows.tile([P, CK], f32, tag="recip_cols")
            nc.vector.reciprocal(recip_cols, ps_scols)
            lk_cols = rows.tile([P, CK], f32, tag="lk_cols")
            nc.vector.tensor_mul(lk_cols, lnr_cols, recip_cols)
            # column -> broadcast: per-chunk identity transposes rebuild the
            # row vector on partition 0, then one rank-1 matmul per half
            # replicates it across all 128 partitions (b_bc[m, c] = lk_c)
            ps_lkrow = ps_mid.tile([1, C], f32, tag="ps_scratch")
            for k in range(CK):
                nc.tensor.transpose(
                    ps_lkrow[:, k * P:(k + 1) * P], lk_cols[:, k:k + 1], ident
                )
            lk_row = rows.tile([1, C], f32, tag="lk_row")
            nc.scalar.copy(lk_row[:, 0:HALF], ps_lkrow[:, 0:HALF])
            nc.vector.tensor_copy(lk_row[:, HALF:], ps_lkrow[:, HALF:])
            ps_bbc = ps_mid.tile([P, C], f32, tag="ps_bbc")
            b_bc = bcpool.tile([P, C], f32, tag="b_bc", bufs=1)
            # gcnt_row[i] = #{c : lk_c > lk_i}: DVE/GPSIMD compares feeding
            # full-rate bf16 ones-matmuls (0/1 exact in bf16), in 512-channel
            # halves so half 1's compares overlap half 0's mask.
            ps_gcnt = ps_mid.tile([1, C], f32, tag="ps_scratch")
            mask_row = rows.tile([1, C], bf16, tag="mask_row")
            mask_bc = bcpool.tile([P, FREE], bf16, tag="mask_bc", bufs=2)
            for h in range(2):
                sl = slice(h * HALF, (h + 1) * HALF)
                nc.tensor.matmul(
                    ps_bbc[:, sl], lhsT=ones_row, rhs=lk_row[:, sl],
                    start=True, stop=True,
                )
                nc.vector.tensor_copy(b_bc[:, sl], ps_bbc[:, sl])
                for q in range(CK):
                    tq = tqpool.tile([P, HALF], bf16, tag="tq", bufs=4)
                    nc.vector.tensor_scalar(
                        tq, b_bc[:, sl], lk_cols[:, q:q + 1], None, op0=ALU.is_lt
                    )
                    nc.tensor.matmul(
                        ps_gcnt[:, sl],
                        lhsT=ones_b,
                        rhs=tq,
                        start=(q == 0),
                        stop=(q == CK - 1),
                    )
                nc.vector.scalar_tensor_tensor(
                    mask_row[:, sl], ps_gcnt[:, sl], float(NKEEP), rng_row[:, sl],
                    op0=ALU.is_lt, op1=ALU.mult,
                )

            # alpha = sum(S) / sum(S * sel), folded into the mask row so
            # pass 2 is a plain 2-operand multiply (runs on GPSIMD too;
            # TensorScalarPtr ops are not legal on the Pool engine)
            scratch = rows.tile([1, C], f32, tag="scratch")
            stats = rows.tile([1, 3], f32, tag="stats")
            nc.vector.scalar_tensor_tensor(
                scratch, ps_gcnt, float(NKEEP), s_row,
                op0=ALU.is_lt, op1=ALU.mult, accum_out=stats[:, 0:1],
            )
            nc.vector.tensor_reduce(stats[:, 1:2], s_row, axis=AXIS.X, op=ALU.add)
            nc.vector.reciprocal(stats[:, 2:3], stats[:, 0:1])
            nc.vector.tensor_scalar(
                stats[:, 2:3], stats[:, 2:3], stats[:, 1:2], None, op0=ALU.mult
            )
            maskf_row = rows.tile([1, C], bf16, tag="maskf_row")
            nc.vector.tensor_scalar(
                maskf_row, mask_row, stats[:, 2:3], None, op0=ALU.mult
            )
            for h in range(2):
                sl = slice(h * HALF, (h + 1) * HALF)
                for g in range(TWO):
                    nc.gpsimd.partition_broadcast(
                        mask_bc[:, g * C + h * HALF:g * C + (h + 1) * HALF],
                        maskf_row[:, sl],
                    )

            # ---- pass 2: out = x * maskf in bf16 (DVE/GPSIMD alternating so
            # DVE keeps casting sample s+1), SWDGE cast-store ----
            for t in range(nt):
                eng = nc.vector if t % 2 == 0 else nc.gpsimd
                eng.tensor_mul(xts[t], xts[t], mask_bc)
                nc.gpsimd.dma_start(out=ot[s, t], in_=xts[t])


def build_nc(ns=NS, hw=HW):
    nc = bacc.Bacc(
        "TRN2", target_bir_lowering=False, debug=False, num_devices=NCORES
    )
    x = nc.dram_tensor("x", [ns, hw, C], f32, kind="ExternalInput").ap()
    r = nc.dram_tensor("r", [ns, C], f32, kind="ExternalInput").ap()
    u = nc.dram_tensor("u", [ns, C], f32, kind="ExternalInput").ap()
    o = nc.dram_tensor("o", [ns, hw, C], f32, kind="ExternalOutput").ap()
    with tile.TileContext(nc) as tc:
        emit(tc, o, x, r, u, ns, hw)
    nc.compile()
    return nc


_cached_nc = None


def kernel(x, r, u):
    global _cached_nc
    if _cached_nc is None:
        _cached_nc = build_nc()
    in_maps = [
        {
            "x": np.ascontiguousarray(x[i * NS:(i + 1) * NS], dtype=np.float32),
            "r": np.ascontiguousarray(r[i * NS:(i + 1) * NS], dtype=np.float32),
            "u": np.ascontiguousarray(u[i * NS:(i + 1) * NS], dtype=np.float32),
        }
        for i in range(NCORES)
    ]
    res = run_bass_kernel_spmd(_cached_nc, in_maps, list(range(NCORES))).results
    return np.concatenate([res[i]["o"] for i in range(NCORES)], axis=0)


# revision 11
# speedup vs baseline: 1.1304x; 1.1304x over previous
"""Trainium2 Bass kernel for DropChannel (topk channel masking).

Math (per sample):
    score_c = mean_hw x[hw, c]                       (only sums needed; 1/HW cancels)
    lk_c    = ln(r_c) * (1 / S_c)                    (log of key r**(1/score); order-preserving)
    gcnt_i  = #{c : lk_c > lk_i}                     (strictly-greater count)
    sel_i   = gcnt_i < C - M                         (identical to thr = sort(key)[C-M]; sel = key >= thr,
                                                      including tie behaviour)
    alpha   = sum(S) / sum(S * sel)
    out     = x * (sel & (u < P)) * alpha

Sharding: pure data parallel, N=32 samples -> 8 cores x 4 samples.

Per-core schedule (4 samples, each [4096, 1024] f32), built so the HBM
stream never idles:
  - x tiles are [128, 2048] (two hw rows per partition line -> 8 KiB
    descriptors; 4 KiB descriptors were read-latency bound)
  - tiles land in a small f32 staging pool; each pair is summed in place
    on the DVE and fp32 score matmuls (needed for bit-exact top-k
    selection) run per pair -- halving fp32 PE work keeps stage recycle
    ahead of the load stream
  - every engine class is load-balanced so no single engine gates the
    pipeline: the f32->bf16 resident casts alternate DVE / ACT, the
    pass-2 multiplies alternate DVE / GPSIMD, and the gcnt compares
    split DVE / GPSIMD -- so sample s+1 keeps loading at full HBM rate
    while sample s runs its mask chain and store burst
  - bf16 residency halves SBUF: TWO full samples fit resident
  - the mask chain uses NO DMAs: row->column and column->row reshapes of
    the tiny score/key vectors run on the PE (K=1 column matmuls,
    per-chunk identity transposes, rank-1 broadcast matmuls); tiny
    SBUF-SBUF DMAs took ~6us each behind 1 MiB load packets
  - pass 2 multiplies the bf16 tiles in place (mask 0/1 bf16 exact,
    alpha an f32 per-partition scalar) and stores via SWDGE DMA with an
    inline bf16->f32 cast (output precision ~3e-3 relative, gate 2e-2)
  - queue separation: x loads on qSP (sync), r/u loads on qACT (scalar),
    stores on SWDGE (gpsimd)
"""

import numpy as np
from contextlib import ExitStack

import concourse.bacc as bacc
import concourse.tile as tile
from concourse import masks, mybir
from concourse.bass_utils import run_bass_kernel_spmd

N, HW, C = 32, 4096, 1024
NCORES = 8
NS = N // NCORES          # samples per core
P = 128                   # partitions
CK = C // P               # 8 channel chunks; channel c = k*128 + p
NKEEP = C - int(0.5 * C)  # gcnt threshold: keep rows with gcnt < 512
PKEEP = 0.9
HALF = 512                # matmul free-dim limit (one PSUM bank)
TWO = 2                   # hw rows per partition line
FREE = TWO * C            # 2048 free dim per tile
NT = HW // (P * TWO)      # 16 tiles per sample

f32 = mybir.dt.float32
bf16 = mybir.dt.bfloat16
ALU = mybir.AluOpType
ACTF = mybir.ActivationFunctionType
AXIS = mybir.AxisListType

# The score sums must stay fp32: the selection boundary sits only ~2e-5
# (relative) away from the threshold, so scores are matmul'd from the f32
# staging tiles before the bf16 cast. The 0/1 comparison tiles are bf16
# (exact for 0/1, full-rate PE matmul).


def emit(tc, o, x, r, u, ns, hw):
    nc = tc.nc
    nt = hw // (P * TWO)
    xt = x.rearrange("s (t p two) c -> s t p (two c)", p=P, two=TWO)
    ot = o.rearrange("s (t p two) c -> s t p (two c)", p=P, two=TWO)
    # column layout: partition p of chunk k holds channel k*128 + p, so a
    # 128-wide slice of a row vector PE-transposes straight into a column
    rck = r.rearrange("s (k p) -> s p k", p=P)

    with ExitStack() as ctx:
        stage = ctx.enter_context(tc.tile_pool(name="stage", bufs=4))
        xpool = ctx.enter_context(tc.tile_pool(name="xpool", bufs=2 * NT))
        tqpool = ctx.enter_context(tc.tile_pool(name="tqpool", bufs=4))
        bcpool = ctx.enter_context(tc.tile_pool(name="bcpool", bufs=2))
        rows = ctx.enter_context(tc.tile_pool(name="rows", bufs=1))
        consts = ctx.enter_context(tc.tile_pool(name="consts", bufs=1))
        ps_s = ctx.enter_context(tc.tile_pool(name="ps_s", bufs=2, space="PSUM"))
        ps_mid = ctx.enter_context(tc.tile_pool(name="ps_mid", bufs=1, space="PSUM"))

        ones_col = consts.tile([P, 1], f32)
        nc.vector.memset(ones_col, 1.0)
        ones_b = consts.tile([P, 1], bf16)
        nc.vector.memset(ones_b, 1.0)
        ones_11 = consts.tile([1, 1], f32)
        nc.vector.memset(ones_11, 1.0)
        ones_row = consts.tile([1, P], f32)
        nc.vector.memset(ones_row, 1.0)
        ident = consts.tile([P, P], f32)
        masks.make_identity(nc, ident)

        for s in range(ns):
            # precompute pieces that do not depend on x (overlap with loads):
            # ln(r) in (p k) layout, and the bernoulli gate row (u < PKEEP)
            lnr_cols = rows.tile([P, CK], f32, tag="lnr_cols", bufs=2)
            nc.scalar.dma_start(out=lnr_cols, in_=rck[s])
            nc.scalar.activation(lnr_cols, lnr_cols, ACTF.Ln)
            rng_row = rows.tile([1, C], f32, tag="rng_row", bufs=2)
            nc.scalar.dma_start(out=rng_row, in_=u[s:s + 1, :])
            nc.vector.tensor_scalar(rng_row, rng_row, PKEEP, None, op0=ALU.is_lt)

            # ---- pass 1: stage f32 tile pairs, pair-add + score matmuls,
            # cast each tile to a bf16 resident copy ----
            ps_score = ps_s.tile([1, C], f32, tag="ps_score")
            xts = []
            for t in range(nt):
                xstage = stage.tile([P, FREE], f32, tag="xstage")
                nc.sync.dma_start(out=xstage, in_=xt[s, t])
                xres = xpool.tile([P, FREE], bf16, tag="xres")
                if t % 2 == 0:
                    nc.vector.tensor_copy(xres, xstage)
                else:
                    nc.scalar.copy(xres, xstage)
                xts.append(xres)
                for g in range(TWO):
                    for h in range(2):
                        nc.tensor.matmul(
                            ps_score[:, h * HALF:(h + 1) * HALF],
                            lhsT=ones_col,
                            rhs=xstage[:, g * C + h * HALF:g * C + (h + 1) * HALF],
                            start=(t == 0 and g == 0),
                            stop=(t == nt - 1 and g == TWO - 1),
                        )

            # ---- mid: selection mask (DMA-free, PE does the reshapes) ----
            s_row = rows.tile([1, C], f32, tag="s_row")
            nc.scalar.copy(s_row[:, 0:HALF], ps_score[:, 0:HALF])
            nc.vector.tensor_copy(s_row[:, HALF:], ps_score[:, HALF:])
            # row -> column: K=1 matmuls drop each 128-wide row chunk into a
            # PSUM column; reciprocal + logkey in column layout (128-way).
            # scols / lkrow / gcnt share one PSUM slot (disjoint lifetimes):
            # 8 banks = score 2x2 + scratch 2 + bbc 2
            ps_scols = ps_mid.tile([P, CK], f32, tag="ps_scratch")
            for k in range(CK):
                nc.tensor.matmul(
                    ps_scols[:, k:k + 1],
                    lhsT=s_row[:, k * P:(k + 1) * P],
                    rhs=ones_11,
                    start=True, stop=True,
                )
            recip_cols = rows.tile([P, CK], f32, tag="recip_cols")
            nc.vector.reciprocal(recip_cols, ps_scols)
            lk_cols = rows.tile([P, CK], f32, tag="lk_cols")
            nc.vector.tensor_mul(lk_cols, lnr_cols, recip_cols)
            # column -> broadcast: per-chunk identity transposes rebuild the
            # row vector on partition 0, then one rank-1 matmul per half
            # replicates it across all 128 partitions (b_bc[m, c] = lk_c)
            ps_lkrow = ps_mid.tile([1, C], f32, tag="ps_scratch")
            for k in range(CK):
                nc.tensor.transpose(
                    ps_lkrow[:, k * P:(k + 1) * P], lk_cols[:, k:k + 1], ident
                )
            lk_row = rows.tile([1, C], f32, tag="lk_row")
            nc.scalar.copy(lk_row[:, 0:HALF], ps_lkrow[:, 0:HALF])
            nc.vector.tensor_copy(lk_row[:, HALF:], ps_lkrow[:, HALF:])
            ps_bbc = ps_mid.tile([P, C], f32, tag="ps_bbc")
            b_bc = bcpool.tile([P, C], f32, tag="b_bc", bufs=1)
            # gcnt_row[i] = #{c : lk_c > lk_i}: DVE/GPSIMD compares feeding
            # full-rate bf16 ones-matmuls (0/1 exact in bf16), in 512-channel
            # halves so half 1's compares overlap half 0's mask.
            ps_gcnt = ps_mid.tile([1, C], f32, tag="ps_scratch")
            mask_row = rows.tile([1, C], bf16, tag="mask_row")
            mask_bc = bcpool.tile([P, FREE], bf16, tag="mask_bc", bufs=2)
            for h in range(2):
                sl = slice(h * HALF, (h + 1) * HALF)
                nc.tensor.matmul(
                    ps_bbc[:, sl], lhsT=ones_row, rhs=lk_row[:, sl],
                    start=True, stop=True,
                )
                nc.vector.tensor_copy(b_bc[:, sl], ps_bbc[:, sl])
                for q in range(CK):
                    tq = tqpool.tile([P, HALF], bf16, tag="tq", bufs=4)
                    nc.vector.tensor_scalar(
                        tq, b_bc[:, sl], lk_cols[:, q:q + 1], None, op0=ALU.is_lt
                    )
                    nc.tensor.matmul(
                        ps_gcnt[:, sl],
                        lhsT=ones_b,
                        rhs=tq,
                        start=(q == 0),
                        stop=(q == CK - 1),
                    )
                nc.vector.scalar_tensor_tensor(
                    mask_row[:, sl], ps_gcnt[:, sl], float(NKEEP), rng_row[:, sl],
                    op0=ALU.is_lt, op1=ALU.mult,
                )

            # alpha = sum(S) / sum(S * sel), folded into the mask row so
            # pass 2 is a plain 2-operand multiply (runs on GPSIMD too;
            # TensorScalarPtr ops are not legal on the Pool engine)
            scratch = rows.tile([1, C], f32, tag="scratch")
            stats = rows.tile([1, 3], f32, tag="stats")
            nc.vector.scalar_tensor_tensor(
                scratch, ps_gcnt, float(NKEEP), s_row,
                op0=ALU.is_lt, op1=ALU.mult, accum_out=stats[:, 0:1],
            )
            nc.vector.tensor_reduce(stats[:, 1:2], s_row, axis=AXIS.X, op=ALU.add)
            nc.vector.reciprocal(stats[:, 2:3], stats[:, 0:1])
            nc.vector.tensor_scalar(
                stats[:, 2:3], stats[:, 2:3], stats[:, 1:2], None, op0=ALU.mult
            )
            maskf_row = rows.tile([1, C], bf16, tag="maskf_row")
            nc.vector.tensor_scalar(
                maskf_row, mask_row, stats[:, 2:3], None, op0=ALU.mult
            )
            for h in range(2):
                sl = slice(h * HALF, (h + 1) * HALF)
                for g in range(TWO):
                    nc.gpsimd.partition_broadcast(
                        mask_bc[:, g * C + h * HALF:g * C + (h + 1) * HALF],
                        maskf_row[:, sl],
                    )

            # ---- pass 2: out = x * maskf in bf16, SWDGE cast-store. The
            # multiplies hide under the HBM-capped store burst (~47us). ----
            for t in range(nt):
                nc.vector.tensor_mul(xts[t], xts[t], mask_bc)
                nc.gpsimd.dma_start(out=ot[s, t], in_=xts[t])


def build_nc(ns=NS, hw=HW):
    nc = bacc.Bacc(
        "TRN2", target_bir_lowering=False, debug=False, num_devices=NCORES
    )
    x = nc.dram_tensor("x", [ns, hw, C], f32, kind="ExternalInput").ap()
    r = nc.dram_tensor("r", [ns, C], f32, kind="ExternalInput").ap()
    u = nc.dram_tensor("u", [ns, C], f32, kind="ExternalInput").ap()
    o = nc.dram_tensor("o", [ns, hw, C], f32, kind="ExternalOutput").ap()
    with tile.TileContext(nc) as tc:
        emit(tc, o, x, r, u, ns, hw)
    nc.compile()
    return nc


_cached_nc = None


def kernel(x, r, u):
    global _cached_nc
    if _cached_nc is None:
        _cached_nc = build_nc()
    in_maps = [
        {
            "x": np.ascontiguousarray(x[i * NS:(i + 1) * NS], dtype=np.float32),
            "r": np.ascontiguousarray(r[i * NS:(i + 1) * NS], dtype=np.float32),
            "u": np.ascontiguousarray(u[i * NS:(i + 1) * NS], dtype=np.float32),
        }
        for i in range(NCORES)
    ]
    res = run_bass_kernel_spmd(_cached_nc, in_maps, list(range(NCORES))).results
    return np.concatenate([res[i]["o"] for i in range(NCORES)], axis=0)


# revision 14
# speedup vs baseline: 1.1452x; 1.0131x over previous
"""Trainium2 Bass kernel for DropChannel (topk channel masking).

Math (per sample):
    score_c = mean_hw x[hw, c]                       (only sums needed; 1/HW cancels)
    lk_c    = ln(r_c) * (1 / S_c)                    (log of key r**(1/score); order-preserving)
    gcnt_i  = #{c : lk_c > lk_i}                     (strictly-greater count)
    sel_i   = gcnt_i < C - M                         (identical to thr = sort(key)[C-M]; sel = key >= thr,
                                                      including tie behaviour)
    alpha   = sum(S) / sum(S * sel)
    out     = x * (sel & (u < P)) * alpha

Sharding: pure data parallel, N=32 samples -> 8 cores x 4 samples.

Per-core schedule (4 samples, each [4096, 1024] f32), built so the HBM
stream never idles:
  - x tiles are [128, 2048] (two hw rows per partition line -> 8 KiB
    descriptors; 4 KiB descriptors were read-latency bound)
  - tiles land in a small f32 staging pool; each pair is summed in place
    on the DVE and fp32 score matmuls (needed for bit-exact top-k
    selection) run per pair -- halving fp32 PE work keeps stage recycle
    ahead of the load stream
  - every engine class is load-balanced so no single engine gates the
    pipeline: the f32->bf16 resident casts alternate DVE / ACT, the
    pass-2 multiplies alternate DVE / GPSIMD, and the gcnt compares
    split DVE / GPSIMD -- so sample s+1 keeps loading at full HBM rate
    while sample s runs its mask chain and store burst
  - bf16 residency halves SBUF: TWO full samples fit resident
  - the mask chain uses NO DMAs: row->column and column->row reshapes of
    the tiny score/key vectors run on the PE (K=1 column matmuls,
    per-chunk identity transposes, rank-1 broadcast matmuls); tiny
    SBUF-SBUF DMAs took ~6us each behind 1 MiB load packets
  - pass 2 multiplies the bf16 tiles in place (mask 0/1 bf16 exact,
    alpha an f32 per-partition scalar) and stores via SWDGE DMA with an
    inline bf16->f32 cast (output precision ~3e-3 relative, gate 2e-2)
  - queue separation: x loads on qSP (sync), r/u loads on qACT (scalar),
    stores on SWDGE (gpsimd)
"""

import numpy as np
from contextlib import ExitStack

import concourse.bacc as bacc
import concourse.tile as tile
from concourse import masks, mybir
from concourse.bass_utils import run_bass_kernel_spmd

N, HW, C = 32, 4096, 1024
NCORES = 8
NS = N // NCORES          # samples per core
P = 128                   # partitions
CK = C // P               # 8 channel chunks; channel c = k*128 + p
NKEEP = C - int(0.5 * C)  # gcnt threshold: keep rows with gcnt < 512
PKEEP = 0.9
HALF = 512                # matmul free-dim limit (one PSUM bank)
TWO = 2                   # hw rows per partition line
FREE = TWO * C            # 2048 free dim per tile
NT = HW // (P * TWO)      # 16 tiles per sample

f32 = mybir.dt.float32
bf16 = mybir.dt.bfloat16
ALU = mybir.AluOpType
ACTF = mybir.ActivationFunctionType
AXIS = mybir.AxisListType

# The score sums must stay fp32: the selection boundary sits only ~2e-5
# (relative) away from the threshold, so scores are matmul'd from the f32
# staging tiles before the bf16 cast. The 0/1 comparison tiles are bf16
# (exact for 0/1, full-rate PE matmul).


def emit(tc, o, x, r, u, ns, hw):
    nc = tc.nc
    nt = hw // (P * TWO)
    xt = x.rearrange("s (t p two) c -> s t p (two c)", p=P, two=TWO)
    ot = o.rearrange("s (t p two) c -> s t p (two c)", p=P, two=TWO)
    # column layout: partition p of chunk k holds channel k*128 + p, so a
    # 128-wide slice of a row vector PE-transposes straight into a column
    rck = r.rearrange("s (k p) -> s p k", p=P)

    with ExitStack() as ctx:
        stage = ctx.enter_context(tc.tile_pool(name="stage", bufs=4))
        xpool = ctx.enter_context(tc.tile_pool(name="xpool", bufs=2 * NT))
        tqpool = ctx.enter_context(tc.tile_pool(name="tqpool", bufs=4))
        bcpool = ctx.enter_context(tc.tile_pool(name="bcpool", bufs=2))
        rows = ctx.enter_context(tc.tile_pool(name="rows", bufs=1))
        consts = ctx.enter_context(tc.tile_pool(name="consts", bufs=1))
        ps_s = ctx.enter_context(tc.tile_pool(name="ps_s", bufs=2, space="PSUM"))
        ps_mid = ctx.enter_context(tc.tile_pool(name="ps_mid", bufs=1, space="PSUM"))

        ones_col = consts.tile([P, 1], f32)
        nc.vector.memset(ones_col, 1.0)
        ones_b = consts.tile([P, 1], bf16)
        nc.vector.memset(ones_b, 1.0)
        ones_11 = consts.tile([1, 1], f32)
        nc.vector.memset(ones_11, 1.0)
        ones_row = consts.tile([1, P], f32)
        nc.vector.memset(ones_row, 1.0)
        ident = consts.tile([P, P], f32)
        masks.make_identity(nc, ident)

        # pass 2 of sample s-1 is emitted interleaved into sample s's load
        # loop, holding back the last HOLD tiles until after the loads: the
        # held-back stores are the HBM work that fills sample s's otherwise
        # load-only mask-chain window (and the final sample's chain is
        # covered by the previous sample's held-back stores).
        HOLD = 6
        prev = None  # (sample_idx, xts, mask_bc) of the previous sample

        def emit_pass2(ps, pxts, pmask, pt):
            nc.vector.tensor_mul(pxts[pt], pxts[pt], pmask)
            nc.gpsimd.dma_start(out=ot[ps, pt], in_=pxts[pt])

        for s in range(ns):
            # precompute pieces that do not depend on x (overlap with loads):
            # ln(r) in (p k) layout, and the bernoulli gate row (u < PKEEP)
            lnr_cols = rows.tile([P, CK], f32, tag="lnr_cols", bufs=2)
            nc.scalar.dma_start(out=lnr_cols, in_=rck[s])
            nc.scalar.activation(lnr_cols, lnr_cols, ACTF.Ln)
            rng_row = rows.tile([1, C], f32, tag="rng_row", bufs=2)
            nc.scalar.dma_start(out=rng_row, in_=u[s:s + 1, :])
            nc.vector.tensor_scalar(rng_row, rng_row, PKEEP, None, op0=ALU.is_lt)

            # ---- pass 1: stage f32 tiles, score matmuls, bf16 cast ----
            ps_score = ps_s.tile([1, C], f32, tag="ps_score")
            xts = []
            for t in range(nt):
                xstage = stage.tile([P, FREE], f32, tag="xstage")
                nc.sync.dma_start(out=xstage, in_=xt[s, t])
                # cast on ACT (1.7us/tile, under the 2.9us load pitch): the
                # stage pool then recycles without touching DVE, so loads
                # keep streaming while DVE runs the mask chain / pass 2
                xres = xpool.tile([P, FREE], bf16, tag="xres")
                nc.scalar.copy(xres, xstage)
                xts.append(xres)
                for g in range(TWO):
                    for h in range(2):
                        nc.tensor.matmul(
                            ps_score[:, h * HALF:(h + 1) * HALF],
                            lhsT=ones_col,
                            rhs=xstage[:, g * C + h * HALF:g * C + (h + 1) * HALF],
                            start=(t == 0 and g == 0),
                            stop=(t == nt - 1 and g == TWO - 1),
                        )
                if prev is not None and t >= HOLD:
                    emit_pass2(prev[0], prev[1], prev[2], t - HOLD)
            if prev is not None:
                for pt in range(nt - HOLD, nt):
                    emit_pass2(prev[0], prev[1], prev[2], pt)

            # ---- mid: selection mask (DMA-free, PE does the reshapes) ----
            s_row = rows.tile([1, C], f32, tag="s_row")
            nc.scalar.copy(s_row[:, 0:HALF], ps_score[:, 0:HALF])
            nc.vector.tensor_copy(s_row[:, HALF:], ps_score[:, HALF:])
            # row -> column: K=1 matmuls drop each 128-wide row chunk into a
            # PSUM column; reciprocal + logkey in column layout (128-way).
            # scols / lkrow / gcnt share one PSUM slot (disjoint lifetimes):
            # 8 banks = score 2x2 + scratch 2 + bbc 2
            ps_scols = ps_mid.tile([P, CK], f32, tag="ps_scratch")
            for k in range(CK):
                nc.tensor.matmul(
                    ps_scols[:, k:k + 1],
                    lhsT=s_row[:, k * P:(k + 1) * P],
                    rhs=ones_11,
                    start=True, stop=True,
                )
            recip_cols = rows.tile([P, CK], f32, tag="recip_cols")
            nc.vector.reciprocal(recip_cols, ps_scols)
            lk_cols = rows.tile([P, CK], f32, tag="lk_cols")
            nc.vector.tensor_mul(lk_cols, lnr_cols, recip_cols)
            # column -> broadcast: per-chunk identity transposes rebuild the
            # row vector on partition 0, then one rank-1 matmul per half
            # replicates it across all 128 partitions (b_bc[m, c] = lk_c)
            ps_lkrow = ps_mid.tile([1, C], f32, tag="ps_scratch")
            for k in range(CK):
                nc.tensor.transpose(
                    ps_lkrow[:, k * P:(k + 1) * P], lk_cols[:, k:k + 1], ident
                )
            lk_row = rows.tile([1, C], f32, tag="lk_row")
            nc.scalar.copy(lk_row[:, 0:HALF], ps_lkrow[:, 0:HALF])
            nc.vector.tensor_copy(lk_row[:, HALF:], ps_lkrow[:, HALF:])
            ps_bbc = ps_mid.tile([P, C], f32, tag="ps_bbc")
            b_bc = bcpool.tile([P, C], f32, tag="b_bc", bufs=1)
            # gcnt_row[i] = #{c : lk_c > lk_i}: DVE/GPSIMD compares feeding
            # full-rate bf16 ones-matmuls (0/1 exact in bf16), in 512-channel
            # halves so half 1's compares overlap half 0's mask.
            ps_gcnt = ps_mid.tile([1, C], f32, tag="ps_scratch")
            mask_row = rows.tile([1, C], bf16, tag="mask_row")
            mask_bc = bcpool.tile([P, FREE], bf16, tag="mask_bc", bufs=2)
            for h in range(2):
                sl = slice(h * HALF, (h + 1) * HALF)
                nc.tensor.matmul(
                    ps_bbc[:, sl], lhsT=ones_row, rhs=lk_row[:, sl],
                    start=True, stop=True,
                )
                nc.vector.tensor_copy(b_bc[:, sl], ps_bbc[:, sl])
                for q in range(CK):
                    tq = tqpool.tile([P, HALF], bf16, tag="tq", bufs=4)
                    nc.vector.tensor_scalar(
                        tq, b_bc[:, sl], lk_cols[:, q:q + 1], None, op0=ALU.is_lt
                    )
                    nc.tensor.matmul(
                        ps_gcnt[:, sl],
                        lhsT=ones_b,
                        rhs=tq,
                        start=(q == 0),
                        stop=(q == CK - 1),
                    )
                nc.vector.scalar_tensor_tensor(
                    mask_row[:, sl], ps_gcnt[:, sl], float(NKEEP), rng_row[:, sl],
                    op0=ALU.is_lt, op1=ALU.mult,
                )

            # alpha = sum(S) / sum(S * sel), folded into the mask row so
            # pass 2 is a plain 2-operand multiply (runs on GPSIMD too;
            # TensorScalarPtr ops are not legal on the Pool engine)
            scratch = rows.tile([1, C], f32, tag="scratch")
            stats = rows.tile([1, 3], f32, tag="stats")
            nc.vector.scalar_tensor_tensor(
                scratch, ps_gcnt, float(NKEEP), s_row,
                op0=ALU.is_lt, op1=ALU.mult, accum_out=stats[:, 0:1],
            )
            nc.vector.tensor_reduce(stats[:, 1:2], s_row, axis=AXIS.X, op=ALU.add)
            nc.vector.reciprocal(stats[:, 2:3], stats[:, 0:1])
            nc.vector.tensor_scalar(
                stats[:, 2:3], stats[:, 2:3], stats[:, 1:2], None, op0=ALU.mult
            )
            maskf_row = rows.tile([1, C], bf16, tag="maskf_row")
            nc.vector.tensor_scalar(
                maskf_row, mask_row, stats[:, 2:3], None, op0=ALU.mult
            )
            for h in range(2):
                sl = slice(h * HALF, (h + 1) * HALF)
                for g in range(TWO):
                    nc.gpsimd.partition_broadcast(
                        mask_bc[:, g * C + h * HALF:g * C + (h + 1) * HALF],
                        maskf_row[:, sl],
                    )

            prev = (s, xts, mask_bc)

        # flush the final sample's pass 2 (its store burst is the tail)
        for pt in range(nt):
            emit_pass2(prev[0], prev[1], prev[2], pt)


def build_nc(ns=NS, hw=HW):
    nc = bacc.Bacc(
        "TRN2", target_bir_lowering=False, debug=False, num_devices=NCORES
    )
    x = nc.dram_tensor("x", [ns, hw, C], f32, kind="ExternalInput").ap()
    r = nc.dram_tensor("r", [ns, C], f32, kind="ExternalInput").ap()
    u = nc.dram_tensor("u", [ns, C], f32, kind="ExternalInput").ap()
    o = nc.dram_tensor("o", [ns, hw, C], f32, kind="ExternalOutput").ap()
    with tile.TileContext(nc) as tc:
        emit(tc, o, x, r, u, ns, hw)
    nc.compile()
    return nc


_cached_nc = None


def kernel(x, r, u):
    global _cached_nc
    if _cached_nc is None:
        _cached_nc = build_nc()
    in_maps = [
        {
            "x": np.ascontiguousarray(x[i * NS:(i + 1) * NS], dtype=np.float32),
            "r": np.ascontiguousarray(r[i * NS:(i + 1) * NS], dtype=np.float32),
            "u": np.ascontiguousarray(u[i * NS:(i + 1) * NS], dtype=np.float32),
        }
        for i in range(NCORES)
    ]
    res = run_bass_kernel_spmd(_cached_nc, in_maps, list(range(NCORES))).results
    return np.concatenate([res[i]["o"] for i in range(NCORES)], axis=0)


# revision 18
# speedup vs baseline: 1.2272x; 1.0716x over previous
"""Trainium2 Bass kernel for DropChannel (topk channel masking).

Math (per sample):
    score_c = mean_hw x[hw, c]                       (only sums needed; 1/HW cancels)
    lk_c    = ln(r_c) * (1 / S_c)                    (log of key r**(1/score); order-preserving)
    gcnt_i  = #{c : lk_c > lk_i}                     (strictly-greater count)
    sel_i   = gcnt_i < C - M                         (identical to thr = sort(key)[C-M]; sel = key >= thr,
                                                      including tie behaviour)
    alpha   = sum(S) / sum(S * sel)
    out     = x * (sel & (u < P)) * alpha

Sharding: pure data parallel, N=32 samples -> 8 cores x 4 samples.

Per-core schedule (4 samples, each [4096, 1024] f32), built so the HBM
stream never idles:
  - x tiles are [128, 2048] (two hw rows per partition line -> 8 KiB
    descriptors; 4 KiB descriptors were read-latency bound)
  - tiles land in a small f32 staging pool; each pair is summed in place
    on the DVE and fp32 score matmuls (needed for bit-exact top-k
    selection) run per pair -- halving fp32 PE work keeps stage recycle
    ahead of the load stream
  - every engine class is load-balanced so no single engine gates the
    pipeline: the f32->bf16 resident casts alternate DVE / ACT, the
    pass-2 multiplies alternate DVE / GPSIMD, and the gcnt compares
    split DVE / GPSIMD -- so sample s+1 keeps loading at full HBM rate
    while sample s runs its mask chain and store burst
  - bf16 residency halves SBUF: TWO full samples fit resident
  - the mask chain uses NO DMAs: row->column and column->row reshapes of
    the tiny score/key vectors run on the PE (K=1 column matmuls,
    per-chunk identity transposes, rank-1 broadcast matmuls); tiny
    SBUF-SBUF DMAs took ~6us each behind 1 MiB load packets
  - pass 2 multiplies the bf16 tiles in place (mask 0/1 bf16 exact,
    alpha an f32 per-partition scalar) and stores via SWDGE DMA with an
    inline bf16->f32 cast (output precision ~3e-3 relative, gate 2e-2)
  - queue separation: x loads on qSP (sync), r/u loads on qACT (scalar),
    stores on SWDGE (gpsimd)
"""

import numpy as np
from contextlib import ExitStack

import concourse.bacc as bacc
import concourse.tile as tile
from concourse import masks, mybir
from concourse.bass_utils import run_bass_kernel_spmd

N, HW, C = 32, 4096, 1024
NCORES = 8
NS = N // NCORES          # samples per core
P = 128                   # partitions
CK = C // P               # 8 channel chunks; channel c = k*128 + p
NKEEP = C - int(0.5 * C)  # gcnt threshold: keep rows with gcnt < 512
PKEEP = 0.9
HALF = 512                # matmul free-dim limit (one PSUM bank)
TWO = 2                   # hw rows per partition line
FREE = TWO * C            # 2048 free dim per tile
NT = HW // (P * TWO)      # 16 tiles per sample

f32 = mybir.dt.float32
bf16 = mybir.dt.bfloat16
ALU = mybir.AluOpType
ACTF = mybir.ActivationFunctionType
AXIS = mybir.AxisListType

# The score sums must stay fp32: the selection boundary sits only ~2e-5
# (relative) away from the threshold, so scores are matmul'd from the f32
# staging tiles before the bf16 cast. The 0/1 comparison tiles are bf16
# (exact for 0/1, full-rate PE matmul).


def emit(tc, o, x, r, u, ns, hw):
    nc = tc.nc
    nt = hw // (P * TWO)
    xt = x.rearrange("s (t p two) c -> s t p (two c)", p=P, two=TWO)
    ot = o.rearrange("s (t p two) c -> s t p (two c)", p=P, two=TWO)
    # column layout: partition p of chunk k holds channel k*128 + p, so a
    # 128-wide slice of a row vector PE-transposes straight into a column
    rck = r.rearrange("s (k p) -> s p k", p=P)

    with ExitStack() as ctx:
        stage = ctx.enter_context(tc.tile_pool(name="stage", bufs=4))
        xpool = ctx.enter_context(tc.tile_pool(name="xpool", bufs=2 * NT))
        tqpool = ctx.enter_context(tc.tile_pool(name="tqpool", bufs=4))
        bcpool = ctx.enter_context(tc.tile_pool(name="bcpool", bufs=2))
        rows = ctx.enter_context(tc.tile_pool(name="rows", bufs=1))
        consts = ctx.enter_context(tc.tile_pool(name="consts", bufs=1))
        ps_s = ctx.enter_context(tc.tile_pool(name="ps_s", bufs=2, space="PSUM"))
        ps_mid = ctx.enter_context(tc.tile_pool(name="ps_mid", bufs=1, space="PSUM"))

        ones_col = consts.tile([P, 1], f32)
        nc.vector.memset(ones_col, 1.0)
        ones_b = consts.tile([P, 1], bf16)
        nc.vector.memset(ones_b, 1.0)
        ones_11 = consts.tile([1, 1], f32)
        nc.vector.memset(ones_11, 1.0)
        ones_row = consts.tile([1, P], f32)
        nc.vector.memset(ones_row, 1.0)
        ident = consts.tile([P, P], f32)
        masks.make_identity(nc, ident)

        HOLD = 6              # stores reserved to fill the next chain window
        held = []             # (sample, xts, tile) of deferred stores

        for s in range(ns):
            # precompute pieces that do not depend on x (overlap with loads):
            # ln(r) in (p k) layout, and the bernoulli gate row (u < PKEEP)
            lnr_cols = rows.tile([P, CK], f32, tag="lnr_cols", bufs=2)
            nc.scalar.dma_start(out=lnr_cols, in_=rck[s])
            nc.scalar.activation(lnr_cols, lnr_cols, ACTF.Ln)
            rng_row = rows.tile([1, C], f32, tag="rng_row", bufs=2)
            nc.scalar.dma_start(out=rng_row, in_=u[s:s + 1, :])
            nc.vector.tensor_scalar(rng_row, rng_row, PKEEP, None, op0=ALU.is_lt)

            # ---- pass 1: stage f32 tile pairs, pair-add + score matmuls,
            # cast each tile to a bf16 resident copy ----
            ps_score = ps_s.tile([1, C], f32, tag="ps_score")
            xts = []
            for t in range(nt):
                xstage = stage.tile([P, FREE], f32, tag="xstage")
                nc.sync.dma_start(out=xstage, in_=xt[s, t])
                # cast on ACT (1.7us/tile, under the 2.9us load pitch): the
                # stage pool then recycles without touching DVE, so loads
                # keep streaming while DVE runs the mask chain / pass 2
                xres = xpool.tile([P, FREE], bf16, tag="xres")
                nc.scalar.copy(xres, xstage)
                xts.append(xres)
                for g in range(TWO):
                    for h in range(2):
                        nc.tensor.matmul(
                            ps_score[:, h * HALF:(h + 1) * HALF],
                            lhsT=ones_col,
                            rhs=xstage[:, g * C + h * HALF:g * C + (h + 1) * HALF],
                            start=(t == 0 and g == 0),
                            stop=(t == nt - 1 and g == TWO - 1),
                        )

            # ---- mid: selection mask (DMA-free, PE does the reshapes) ----
            s_row = rows.tile([1, C], f32, tag="s_row")
            nc.scalar.copy(s_row[:, 0:HALF], ps_score[:, 0:HALF])
            nc.vector.tensor_copy(s_row[:, HALF:], ps_score[:, HALF:])
            # release the previous sample's held-back stores HERE: a tiny
            # gpsimd op depending on this sample's s_row blocks the gpsimd
            # FIFO, so those stores execute exactly during this otherwise
            # load-only mask-chain window instead of draining earlier.
            if held:
                gate = rows.tile([1, 1], f32, tag="gate", bufs=2)
                nc.gpsimd.partition_broadcast(gate, s_row[:, 0:1])
                for ps, pxts, pt in held:
                    nc.gpsimd.dma_start(out=ot[ps, pt], in_=pxts[pt])
                held = []
            # row -> column: K=1 matmuls drop each 128-wide row chunk into a
            # PSUM column; reciprocal + logkey in column layout (128-way).
            # scols / lkrow / gcnt share one PSUM slot (disjoint lifetimes):
            # 8 banks = score 2x2 + scratch 2 + bbc 2
            ps_scols = ps_mid.tile([P, CK], f32, tag="ps_scratch")
            for k in range(CK):
                nc.tensor.matmul(
                    ps_scols[:, k:k + 1],
                    lhsT=s_row[:, k * P:(k + 1) * P],
                    rhs=ones_11,
                    start=True, stop=True,
                )
            recip_cols = rows.tile([P, CK], f32, tag="recip_cols")
            nc.vector.reciprocal(recip_cols, ps_scols)
            lk_cols = rows.tile([P, CK], f32, tag="lk_cols")
            nc.vector.tensor_mul(lk_cols, lnr_cols, recip_cols)
            # column -> broadcast: per-chunk identity transposes rebuild the
            # row vector on partition 0, then one rank-1 matmul per half
            # replicates it across all 128 partitions (b_bc[m, c] = lk_c)
            ps_lkrow = ps_mid.tile([1, C], f32, tag="ps_scratch")
            for k in range(CK):
                nc.tensor.transpose(
                    ps_lkrow[:, k * P:(k + 1) * P], lk_cols[:, k:k + 1], ident
                )
            lk_row = rows.tile([1, C], f32, tag="lk_row")
            nc.scalar.copy(lk_row[:, 0:HALF], ps_lkrow[:, 0:HALF])
            nc.vector.tensor_copy(lk_row[:, HALF:], ps_lkrow[:, HALF:])
            ps_bbc = ps_mid.tile([P, C], f32, tag="ps_bbc")
            b_bc = bcpool.tile([P, C], f32, tag="b_bc", bufs=1)
            # gcnt_row[i] = #{c : lk_c > lk_i}: DVE/GPSIMD compares feeding
            # full-rate bf16 ones-matmuls (0/1 exact in bf16), in 512-channel
            # halves so half 1's compares overlap half 0's mask.
            ps_gcnt = ps_mid.tile([1, C], f32, tag="ps_scratch")
            mask_row = rows.tile([1, C], bf16, tag="mask_row")
            mask_bc = bcpool.tile([P, FREE], bf16, tag="mask_bc", bufs=2)
            for h in range(2):
                sl = slice(h * HALF, (h + 1) * HALF)
                nc.tensor.matmul(
                    ps_bbc[:, sl], lhsT=ones_row, rhs=lk_row[:, sl],
                    start=True, stop=True,
                )
                nc.vector.tensor_copy(b_bc[:, sl], ps_bbc[:, sl])
                for q in range(CK):
                    tq = tqpool.tile([P, HALF], bf16, tag="tq", bufs=4)
                    nc.vector.tensor_scalar(
                        tq, b_bc[:, sl], lk_cols[:, q:q + 1], None, op0=ALU.is_lt
                    )
                    nc.tensor.matmul(
                        ps_gcnt[:, sl],
                        lhsT=ones_b,
                        rhs=tq,
                        start=(q == 0),
                        stop=(q == CK - 1),
                    )
                nc.vector.scalar_tensor_tensor(
                    mask_row[:, sl], ps_gcnt[:, sl], float(NKEEP), rng_row[:, sl],
                    op0=ALU.is_lt, op1=ALU.mult,
                )

            # alpha = sum(S) / sum(S * sel), folded into the mask row so
            # pass 2 is a plain 2-operand multiply (runs on GPSIMD too;
            # TensorScalarPtr ops are not legal on the Pool engine)
            scratch = rows.tile([1, C], f32, tag="scratch")
            stats = rows.tile([1, 3], f32, tag="stats")
            nc.vector.scalar_tensor_tensor(
                scratch, ps_gcnt, float(NKEEP), s_row,
                op0=ALU.is_lt, op1=ALU.mult, accum_out=stats[:, 0:1],
            )
            nc.vector.tensor_reduce(stats[:, 1:2], s_row, axis=AXIS.X, op=ALU.add)
            nc.vector.reciprocal(stats[:, 2:3], stats[:, 0:1])
            nc.vector.tensor_scalar(
                stats[:, 2:3], stats[:, 2:3], stats[:, 1:2], None, op0=ALU.mult
            )
            maskf_row = rows.tile([1, C], bf16, tag="maskf_row")
            nc.vector.tensor_scalar(
                maskf_row, mask_row, stats[:, 2:3], None, op0=ALU.mult
            )
            for h in range(2):
                sl = slice(h * HALF, (h + 1) * HALF)
                for g in range(TWO):
                    nc.gpsimd.partition_broadcast(
                        mask_bc[:, g * C + h * HALF:g * C + (h + 1) * HALF],
                        maskf_row[:, sl],
                    )

            # ---- pass 2: out = x * maskf in bf16, SWDGE cast-store. The
            # multiplies hide under the HBM-capped store burst. The last
            # HOLD tiles' stores are deferred to the next sample's chain
            # window (released by the gate above). ----
            for t in range(nt):
                nc.vector.tensor_mul(xts[t], xts[t], mask_bc)
                if t < nt - HOLD:
                    nc.gpsimd.dma_start(out=ot[s, t], in_=xts[t])
                else:
                    held.append((s, xts, t))

        # flush the final sample's held stores
        for ps, pxts, pt in held:
            nc.gpsimd.dma_start(out=ot[ps, pt], in_=pxts[pt])


def build_nc(ns=NS, hw=HW):
    nc = bacc.Bacc(
        "TRN2", target_bir_lowering=False, debug=False, num_devices=NCORES
    )
    x = nc.dram_tensor("x", [ns, hw, C], f32, kind="ExternalInput").ap()
    r = nc.dram_tensor("r", [ns, C], f32, kind="ExternalInput").ap()
    u = nc.dram_tensor("u", [ns, C], f32, kind="ExternalInput").ap()
    o = nc.dram_tensor("o", [ns, hw, C], f32, kind="ExternalOutput").ap()
    with tile.TileContext(nc) as tc:
        emit(tc, o, x, r, u, ns, hw)
    nc.compile()
    return nc


_cached_nc = None


def kernel(x, r, u):
    global _cached_nc
    if _cached_nc is None:
        _cached_nc = build_nc()
    in_maps = [
        {
            "x": np.ascontiguousarray(x[i * NS:(i + 1) * NS], dtype=np.float32),
            "r": np.ascontiguousarray(r[i * NS:(i + 1) * NS], dtype=np.float32),
            "u": np.ascontiguousarray(u[i * NS:(i + 1) * NS], dtype=np.float32),
        }
        for i in range(NCORES)
    ]
    res = run_bass_kernel_spmd(_cached_nc, in_maps, list(range(NCORES))).results
    return np.concatenate([res[i]["o"] for i in range(NCORES)], axis=0)


# revision 22
# speedup vs baseline: 1.2638x; 1.0298x over previous
"""Trainium2 Bass kernel for DropChannel (topk channel masking).

Math (per sample):
    score_c = mean_hw x[hw, c]                       (only sums needed; 1/HW cancels)
    lk_c    = ln(r_c) * (1 / S_c)                    (log of key r**(1/score); order-preserving)
    gcnt_i  = #{c : lk_c > lk_i}                     (strictly-greater count)
    sel_i   = gcnt_i < C - M                         (identical to thr = sort(key)[C-M]; sel = key >= thr,
                                                      including tie behaviour)
    alpha   = sum(S) / sum(S * sel)
    out     = x * (sel & (u < P)) * alpha

Sharding: pure data parallel, N=32 samples -> 8 cores x 4 samples.

Per-core schedule (4 samples, each [4096, 1024] f32), built so the HBM
stream never idles:
  - x tiles are [128, 2048] (two hw rows per partition line -> 8 KiB
    descriptors; 4 KiB descriptors were read-latency bound)
  - tiles land in a small f32 staging pool; each pair is summed in place
    on the DVE and fp32 score matmuls (needed for bit-exact top-k
    selection) run per pair -- halving fp32 PE work keeps stage recycle
    ahead of the load stream
  - every engine class is load-balanced so no single engine gates the
    pipeline: the f32->bf16 resident casts alternate DVE / ACT, the
    pass-2 multiplies alternate DVE / GPSIMD, and the gcnt compares
    split DVE / GPSIMD -- so sample s+1 keeps loading at full HBM rate
    while sample s runs its mask chain and store burst
  - bf16 residency halves SBUF: TWO full samples fit resident
  - the mask chain uses NO DMAs: row->column and column->row reshapes of
    the tiny score/key vectors run on the PE (K=1 column matmuls,
    per-chunk identity transposes, rank-1 broadcast matmuls); tiny
    SBUF-SBUF DMAs took ~6us each behind 1 MiB load packets
  - pass 2 multiplies the bf16 tiles in place (mask 0/1 bf16 exact,
    alpha an f32 per-partition scalar) and stores via SWDGE DMA with an
    inline bf16->f32 cast (output precision ~3e-3 relative, gate 2e-2)
  - queue separation: x loads on qSP (sync), r/u loads on qACT (scalar),
    stores on SWDGE (gpsimd)
"""

import numpy as np
from contextlib import ExitStack

import concourse.bacc as bacc
import concourse.tile as tile
from concourse import masks, mybir
from concourse.bass_utils import run_bass_kernel_spmd

N, HW, C = 32, 4096, 1024
NCORES = 8
NS = N // NCORES          # samples per core
P = 128                   # partitions
CK = C // P               # 8 channel chunks; channel c = k*128 + p
NKEEP = C - int(0.5 * C)  # gcnt threshold: keep rows with gcnt < 512
PKEEP = 0.9
HALF = 512                # matmul free-dim limit (one PSUM bank)
TWO = 2                   # hw rows per partition line
FREE = TWO * C            # 2048 free dim per tile
NT = HW // (P * TWO)      # 16 tiles per sample

f32 = mybir.dt.float32
bf16 = mybir.dt.bfloat16
ALU = mybir.AluOpType
ACTF = mybir.ActivationFunctionType
AXIS = mybir.AxisListType

# The score sums must stay fp32: the selection boundary sits only ~2e-5
# (relative) away from the threshold, so scores are matmul'd from the f32
# staging tiles before the bf16 cast. The 0/1 comparison tiles are bf16
# (exact for 0/1, full-rate PE matmul).


def emit(tc, o, x, r, u, ns, hw):
    nc = tc.nc
    nt = hw // (P * TWO)
    xt = x.rearrange("s (t p two) c -> s t p (two c)", p=P, two=TWO)
    ot = o.rearrange("s (t p two) c -> s t p (two c)", p=P, two=TWO)
    # column layout: partition p of chunk k holds channel k*128 + p, so a
    # 128-wide slice of a row vector PE-transposes straight into a column
    rck = r.rearrange("s (k p) -> s p k", p=P)

    with ExitStack() as ctx:
        stage = ctx.enter_context(tc.tile_pool(name="stage", bufs=5))
        xpool = ctx.enter_context(tc.tile_pool(name="xpool", bufs=2 * NT))
        tqpool = ctx.enter_context(tc.tile_pool(name="tqpool", bufs=4))
        bcpool = ctx.enter_context(tc.tile_pool(name="bcpool", bufs=2))
        rows = ctx.enter_context(tc.tile_pool(name="rows", bufs=1))
        consts = ctx.enter_context(tc.tile_pool(name="consts", bufs=1))
        ps_s = ctx.enter_context(tc.tile_pool(name="ps_s", bufs=2, space="PSUM"))
        ps_mid = ctx.enter_context(tc.tile_pool(name="ps_mid", bufs=1, space="PSUM"))

        ones_col = consts.tile([P, 1], f32)
        nc.vector.memset(ones_col, 1.0)
        ones_b = consts.tile([P, 1], bf16)
        nc.vector.memset(ones_b, 1.0)
        ones_11 = consts.tile([1, 1], f32)
        nc.vector.memset(ones_11, 1.0)
        ones_row = consts.tile([1, P], f32)
        nc.vector.memset(ones_row, 1.0)
        ident = consts.tile([P, P], f32)
        masks.make_identity(nc, ident)

        HOLD = 6              # stores reserved to fill the next chain window
        held = []             # (sample, xts, tile) of deferred stores

        for s in range(ns):
            # precompute pieces that do not depend on x (overlap with loads):
            # ln(r) in (p k) layout, and the bernoulli gate row (u < PKEEP)
            lnr_cols = rows.tile([P, CK], f32, tag="lnr_cols", bufs=2)
            nc.scalar.dma_start(out=lnr_cols, in_=rck[s])
            nc.scalar.activation(lnr_cols, lnr_cols, ACTF.Ln)
            rng_row = rows.tile([1, C], f32, tag="rng_row", bufs=2)
            nc.scalar.dma_start(out=rng_row, in_=u[s:s + 1, :])
            nc.vector.tensor_scalar(rng_row, rng_row, PKEEP, None, op0=ALU.is_lt)

            # ---- pass 1: stage f32 tile pairs, pair-add + score matmuls,
            # cast each tile to a bf16 resident copy ----
            ps_score = ps_s.tile([1, C], f32, tag="ps_score")
            xts = []
            for t in range(nt):
                xstage = stage.tile([P, FREE], f32, tag="xstage")
                nc.sync.dma_start(out=xstage, in_=xt[s, t])
                # cast on ACT (1.7us/tile, under the 2.9us load pitch): the
                # stage pool then recycles without touching DVE, so loads
                # keep streaming while DVE runs the mask chain / pass 2
                xres = xpool.tile([P, FREE], bf16, tag="xres")
                nc.scalar.copy(xres, xstage)
                xts.append(xres)
                for g in range(TWO):
                    for h in range(2):
                        nc.tensor.matmul(
                            ps_score[:, h * HALF:(h + 1) * HALF],
                            lhsT=ones_col,
                            rhs=xstage[:, g * C + h * HALF:g * C + (h + 1) * HALF],
                            start=(t == 0 and g == 0),
                            stop=(t == nt - 1 and g == TWO - 1),
                        )

            # ---- mid: selection mask (DMA-free, PE does the reshapes).
            # All chain copies stay on DVE: the ACT queue must hold ONLY
            # casts, or the next sample's casts (and with them the whole
            # load stream) block behind chain-dependent ACT ops. ----
            s_row = rows.tile([1, C], f32, tag="s_row")
            nc.vector.tensor_copy(s_row, ps_score)
            # release the previous sample's held-back stores HERE: a tiny
            # gpsimd op depending on this sample's s_row blocks the gpsimd
            # FIFO, so those stores execute exactly during this otherwise
            # load-only mask-chain window instead of draining earlier.
            if held:
                gate = rows.tile([1, 1], f32, tag="gate", bufs=2)
                nc.gpsimd.partition_broadcast(gate, s_row[:, 0:1])
                for ps, pxts, pt in held:
                    nc.gpsimd.dma_start(out=ot[ps, pt], in_=pxts[pt])
                held = []
            # row -> column: K=1 matmuls drop each 128-wide row chunk into a
            # PSUM column; reciprocal + logkey in column layout (128-way).
            # scols / lkrow / gcnt share one PSUM slot (disjoint lifetimes):
            # 8 banks = score 2x2 + scratch 2 + bbc 2
            ps_scols = ps_mid.tile([P, CK], f32, tag="ps_scratch")
            for k in range(CK):
                nc.tensor.matmul(
                    ps_scols[:, k:k + 1],
                    lhsT=s_row[:, k * P:(k + 1) * P],
                    rhs=ones_11,
                    start=True, stop=True,
                )
            recip_cols = rows.tile([P, CK], f32, tag="recip_cols")
            nc.vector.reciprocal(recip_cols, ps_scols)
            lk_cols = rows.tile([P, CK], f32, tag="lk_cols")
            nc.vector.tensor_mul(lk_cols, lnr_cols, recip_cols)
            # column -> broadcast: per-chunk identity transposes rebuild the
            # row vector on partition 0, then one rank-1 matmul per half
            # replicates it across all 128 partitions (b_bc[m, c] = lk_c)
            ps_lkrow = ps_mid.tile([1, C], f32, tag="ps_scratch")
            for k in range(CK):
                nc.tensor.transpose(
                    ps_lkrow[:, k * P:(k + 1) * P], lk_cols[:, k:k + 1], ident
                )
            lk_row = rows.tile([1, C], f32, tag="lk_row")
            nc.vector.tensor_copy(lk_row, ps_lkrow)
            ps_bbc = ps_mid.tile([P, C], f32, tag="ps_bbc")
            b_bc = bcpool.tile([P, C], f32, tag="b_bc", bufs=1)
            # gcnt_row[i] = #{c : lk_c > lk_i}: DVE/GPSIMD compares feeding
            # full-rate bf16 ones-matmuls (0/1 exact in bf16), in 512-channel
            # halves so half 1's compares overlap half 0's mask.
            ps_gcnt = ps_mid.tile([1, C], f32, tag="ps_scratch")
            mask_row = rows.tile([1, C], bf16, tag="mask_row")
            mask_bc = bcpool.tile([P, FREE], bf16, tag="mask_bc", bufs=2)
            for h in range(2):
                sl = slice(h * HALF, (h + 1) * HALF)
                nc.tensor.matmul(
                    ps_bbc[:, sl], lhsT=ones_row, rhs=lk_row[:, sl],
                    start=True, stop=True,
                )
                nc.vector.tensor_copy(b_bc[:, sl], ps_bbc[:, sl])
                for q in range(CK):
                    tq = tqpool.tile([P, HALF], bf16, tag="tq", bufs=4)
                    nc.vector.tensor_scalar(
                        tq, b_bc[:, sl], lk_cols[:, q:q + 1], None, op0=ALU.is_lt
                    )
                    nc.tensor.matmul(
                        ps_gcnt[:, sl],
                        lhsT=ones_b,
                        rhs=tq,
                        start=(q == 0),
                        stop=(q == CK - 1),
                    )
                nc.vector.scalar_tensor_tensor(
                    mask_row[:, sl], ps_gcnt[:, sl], float(NKEEP), rng_row[:, sl],
                    op0=ALU.is_lt, op1=ALU.mult,
                )

            # alpha = sum(S) / sum(S * sel), folded into the mask row so
            # pass 2 is a plain 2-operand multiply. rng_row doubles as the
            # scratch output (its gate values were consumed by mask_row).
            stats = rows.tile([1, 3], f32, tag="stats")
            nc.vector.scalar_tensor_tensor(
                rng_row, ps_gcnt, float(NKEEP), s_row,
                op0=ALU.is_lt, op1=ALU.mult, accum_out=stats[:, 0:1],
            )
            nc.vector.tensor_reduce(stats[:, 1:2], s_row, axis=AXIS.X, op=ALU.add)
            nc.vector.reciprocal(stats[:, 2:3], stats[:, 0:1])
            nc.vector.tensor_scalar(
                stats[:, 2:3], stats[:, 2:3], stats[:, 1:2], None, op0=ALU.mult
            )
            maskf_row = rows.tile([1, C], bf16, tag="maskf_row")
            nc.vector.tensor_scalar(
                maskf_row, mask_row, stats[:, 2:3], None, op0=ALU.mult
            )
            for h in range(2):
                sl = slice(h * HALF, (h + 1) * HALF)
                for g in range(TWO):
                    nc.gpsimd.partition_broadcast(
                        mask_bc[:, g * C + h * HALF:g * C + (h + 1) * HALF],
                        maskf_row[:, sl],
                    )

            # ---- pass 2: out = x * maskf in bf16, SWDGE cast-store. The
            # multiplies hide under the HBM-capped store burst. The last
            # HOLD tiles' stores are deferred to the next sample's chain
            # window (released by the gate above). ----
            for t in range(nt):
                nc.vector.tensor_mul(xts[t], xts[t], mask_bc)
                if t < nt - HOLD:
                    nc.gpsimd.dma_start(out=ot[s, t], in_=xts[t])
                else:
                    held.append((s, xts, t))

        # flush the final sample's held stores
        for ps, pxts, pt in held:
            nc.gpsimd.dma_start(out=ot[ps, pt], in_=pxts[pt])


def build_nc(ns=NS, hw=HW):
    nc = bacc.Bacc(
        "TRN2", target_bir_lowering=False, debug=False, num_devices=NCORES
    )
    x = nc.dram_tensor("x", [ns, hw, C], f32, kind="ExternalInput").ap()
    r = nc.dram_tensor("r", [ns, C], f32, kind="ExternalInput").ap()
    u = nc.dram_tensor("u", [ns, C], f32, kind="ExternalInput").ap()
    o = nc.dram_tensor("o", [ns, hw, C], f32, kind="ExternalOutput").ap()
    with tile.TileContext(nc) as tc:
        emit(tc, o, x, r, u, ns, hw)
    nc.compile()
    return nc


_cached_nc = None


def kernel(x, r, u):
    global _cached_nc
    if _cached_nc is None:
        _cached_nc = build_nc()
    in_maps = [
        {
            "x": np.ascontiguousarray(x[i * NS:(i + 1) * NS], dtype=np.float32),
            "r": np.ascontiguousarray(r[i * NS:(i + 1) * NS], dtype=np.float32),
            "u": np.ascontiguousarray(u[i * NS:(i + 1) * NS], dtype=np.float32),
        }
        for i in range(NCORES)
    ]
    res = run_bass_kernel_spmd(_cached_nc, in_maps, list(range(NCORES))).results
    return np.concatenate([res[i]["o"] for i in range(NCORES)], axis=0)


# revision 25
# speedup vs baseline: 1.2749x; 1.0087x over previous
"""Trainium2 Bass kernel for DropChannel (topk channel masking).

Math (per sample):
    score_c = mean_hw x[hw, c]                       (only sums needed; 1/HW cancels)
    lk_c    = ln(r_c) * (1 / S_c)                    (log of key r**(1/score); order-preserving)
    gcnt_i  = #{c : lk_c > lk_i}                     (strictly-greater count)
    sel_i   = gcnt_i < C - M                         (identical to thr = sort(key)[C-M]; sel = key >= thr,
                                                      including tie behaviour)
    alpha   = sum(S) / sum(S * sel)
    out     = x * (sel & (u < P)) * alpha

Sharding: pure data parallel, N=32 samples -> 8 cores x 4 samples.

Per-core schedule (4 samples, each [4096, 1024] f32), built so the HBM
stream never idles:
  - x tiles are [128, 2048] (two hw rows per partition line -> 8 KiB
    descriptors; 4 KiB descriptors were read-latency bound at ~78%)
  - tiles land in a deep f32 staging pool; fp32 score matmuls (needed
    for bit-exact top-k selection) read the staged tile and an ACT-only
    f32->bf16 cast makes the resident copy. ACT is reserved exclusively
    for casts: stage recycle then never touches DVE, so sample s+1 keeps
    loading at the full HBM rate while DVE runs sample s's mask chain
    and pass-2 multiplies. The deep staging rides out the PE-sequencer
    FIFO jam while chain matmuls wait on chain DVE ops.
  - bf16 residency halves SBUF: TWO full samples fit resident
  - the mask chain uses NO DMAs: row->column and column->row reshapes of
    the tiny score/key vectors run on the PE (K=1 column matmuls,
    per-chunk identity transposes, rank-1 broadcast matmuls); tiny
    SBUF-SBUF DMAs took ~6us each behind 1 MiB load packets
  - alpha is folded into the bf16 mask row before the gpsimd partition
    broadcasts, so pass 2 is one in-place DVE multiply per tile followed
    by a SWDGE store with inline bf16->f32 cast (output precision ~3e-3
    relative, harness gate 2e-2)
  - the last HOLD stores of each sample are deferred and released behind
    a gpsimd gate op that depends on the NEXT sample's chain start: the
    gpsimd FIFO holds them until exactly the load-free chain window, so
    that window still has HBM work
  - queue separation: x loads on qSP (sync), r/u loads on qACT (scalar),
    stores on SWDGE (gpsimd)
"""

import numpy as np
from contextlib import ExitStack

import concourse.bacc as bacc
import concourse.tile as tile
from concourse import masks, mybir
from concourse.bass_utils import run_bass_kernel_spmd

N, HW, C = 32, 4096, 1024
NCORES = 8
NS = N // NCORES          # samples per core
P = 128                   # partitions
CK = C // P               # 8 channel chunks; channel c = k*128 + p
NKEEP = C - int(0.5 * C)  # gcnt threshold: keep rows with gcnt < 512
PKEEP = 0.9
HALF = 512                # matmul free-dim limit (one PSUM bank)
TWO = 2                   # hw rows per partition line
FREE = TWO * C            # 2048 free dim per tile
NT = HW // (P * TWO)      # 16 tiles per sample

f32 = mybir.dt.float32
bf16 = mybir.dt.bfloat16
ALU = mybir.AluOpType
ACTF = mybir.ActivationFunctionType
AXIS = mybir.AxisListType

# The score sums must stay fp32: the selection boundary sits only ~2e-5
# (relative) away from the threshold, so scores are matmul'd from the f32
# staging tiles before the bf16 cast. The 0/1 comparison tiles are bf16
# (exact for 0/1, full-rate PE matmul).


def emit(tc, o, x, r, u, ns, hw):
    nc = tc.nc
    nt = hw // (P * TWO)
    xt = x.rearrange("s (t p two) c -> s t p (two c)", p=P, two=TWO)
    ot = o.rearrange("s (t p two) c -> s t p (two c)", p=P, two=TWO)
    # column layout: partition p of chunk k holds channel k*128 + p, so a
    # 128-wide slice of a row vector PE-transposes straight into a column
    rck = r.rearrange("s (k p) -> s p k", p=P)

    with ExitStack() as ctx:
        stage = ctx.enter_context(tc.tile_pool(name="stage", bufs=5))
        xpool = ctx.enter_context(tc.tile_pool(name="xpool", bufs=2 * NT))
        tqpool = ctx.enter_context(tc.tile_pool(name="tqpool", bufs=3))
        bcpool = ctx.enter_context(tc.tile_pool(name="bcpool", bufs=2))
        rows = ctx.enter_context(tc.tile_pool(name="rows", bufs=1))
        consts = ctx.enter_context(tc.tile_pool(name="consts", bufs=1))
        ps_s = ctx.enter_context(tc.tile_pool(name="ps_s", bufs=2, space="PSUM"))
        ps_mid = ctx.enter_context(tc.tile_pool(name="ps_mid", bufs=1, space="PSUM"))

        ones_col = consts.tile([P, 1], f32)
        nc.vector.memset(ones_col, 1.0)
        ones_b = consts.tile([P, 1], bf16)
        nc.vector.memset(ones_b, 1.0)
        ones_11 = consts.tile([1, 1], f32)
        nc.vector.memset(ones_11, 1.0)
        ones_row = consts.tile([1, P], f32)
        nc.vector.memset(ones_row, 1.0)
        ident = consts.tile([P, P], f32)
        masks.make_identity(nc, ident)

        HOLD = 8              # stores reserved to fill the next chain window
        held = []             # (sample, xts, tile) of deferred stores

        for s in range(ns):
            # precompute pieces that do not depend on x (overlap with loads):
            # ln(r) in (p k) layout, and the bernoulli gate row (u < PKEEP)
            lnr_cols = rows.tile([P, CK], f32, tag="lnr_cols", bufs=2)
            nc.scalar.dma_start(out=lnr_cols, in_=rck[s])
            nc.scalar.activation(lnr_cols, lnr_cols, ACTF.Ln)
            rng_row = rows.tile([1, C], f32, tag="rng_row", bufs=2)
            nc.scalar.dma_start(out=rng_row, in_=u[s:s + 1, :])
            nc.vector.tensor_scalar(rng_row, rng_row, PKEEP, None, op0=ALU.is_lt)

            # ---- pass 1: stage f32 tile pairs, pair-add + score matmuls,
            # cast each tile to a bf16 resident copy ----
            ps_score = ps_s.tile([1, C], f32, tag="ps_score")
            xts = []
            for t in range(nt):
                xstage = stage.tile([P, FREE], f32, tag="xstage")
                nc.sync.dma_start(out=xstage, in_=xt[s, t])
                # cast on ACT (1.7us/tile, under the 2.9us load pitch): the
                # stage pool then recycles without touching DVE, so loads
                # keep streaming while DVE runs the mask chain / pass 2
                xres = xpool.tile([P, FREE], bf16, tag="xres")
                nc.scalar.copy(xres, xstage)
                xts.append(xres)
                for g in range(TWO):
                    for h in range(2):
                        nc.tensor.matmul(
                            ps_score[:, h * HALF:(h + 1) * HALF],
                            lhsT=ones_col,
                            rhs=xstage[:, g * C + h * HALF:g * C + (h + 1) * HALF],
                            start=(t == 0 and g == 0),
                            stop=(t == nt - 1 and g == TWO - 1),
                        )

            # ---- mid: selection mask (DMA-free, PE does the reshapes).
            # All chain copies stay on DVE: the ACT queue must hold ONLY
            # casts, or the next sample's casts (and with them the whole
            # load stream) block behind chain-dependent ACT ops. ----
            s_row = rows.tile([1, C], f32, tag="s_row")
            nc.vector.tensor_copy(s_row, ps_score)
            # release the previous sample's held-back stores HERE: a tiny
            # gpsimd op depending on this sample's s_row blocks the gpsimd
            # FIFO, so those stores execute exactly during this otherwise
            # load-only mask-chain window instead of draining earlier.
            if held:
                gate = rows.tile([1, 1], f32, tag="gate", bufs=2)
                nc.gpsimd.partition_broadcast(gate, s_row[:, 0:1])
                for ps, pxts, pt in held:
                    nc.gpsimd.dma_start(out=ot[ps, pt], in_=pxts[pt])
                held = []
            # row -> column: K=1 matmuls drop each 128-wide row chunk into a
            # PSUM column; reciprocal + logkey in column layout (128-way).
            # scols / lkrow / gcnt share one PSUM slot (disjoint lifetimes):
            # 8 banks = score 2x2 + scratch 2 + bbc 2
            ps_scols = ps_mid.tile([P, CK], f32, tag="ps_scratch")
            for k in range(CK):
                nc.tensor.matmul(
                    ps_scols[:, k:k + 1],
                    lhsT=s_row[:, k * P:(k + 1) * P],
                    rhs=ones_11,
                    start=True, stop=True,
                )
            recip_cols = rows.tile([P, CK], f32, tag="recip_cols")
            nc.vector.reciprocal(recip_cols, ps_scols)
            lk_cols = rows.tile([P, CK], f32, tag="lk_cols")
            nc.vector.tensor_mul(lk_cols, lnr_cols, recip_cols)
            # column -> broadcast: per-chunk identity transposes rebuild the
            # row vector on partition 0, then one rank-1 matmul per half
            # replicates it across all 128 partitions (b_bc[m, c] = lk_c)
            ps_lkrow = ps_mid.tile([1, C], f32, tag="ps_scratch")
            for k in range(CK):
                nc.tensor.transpose(
                    ps_lkrow[:, k * P:(k + 1) * P], lk_cols[:, k:k + 1], ident
                )
            lk_row = rows.tile([1, C], f32, tag="lk_row")
            nc.vector.tensor_copy(lk_row, ps_lkrow)
            ps_bbc = ps_mid.tile([P, C], f32, tag="ps_bbc")
            b_bc = bcpool.tile([P, C], f32, tag="b_bc", bufs=1)
            # gcnt_row[i] = #{c : lk_c > lk_i}: DVE/GPSIMD compares feeding
            # full-rate bf16 ones-matmuls (0/1 exact in bf16), in 512-channel
            # halves so half 1's compares overlap half 0's mask.
            ps_gcnt = ps_mid.tile([1, C], f32, tag="ps_scratch")
            mask_row = rows.tile([1, C], bf16, tag="mask_row")
            mask_bc = bcpool.tile([P, FREE], bf16, tag="mask_bc", bufs=2)
            for h in range(2):
                sl = slice(h * HALF, (h + 1) * HALF)
                nc.tensor.matmul(
                    ps_bbc[:, sl], lhsT=ones_row, rhs=lk_row[:, sl],
                    start=True, stop=True,
                )
                nc.vector.tensor_copy(b_bc[:, sl], ps_bbc[:, sl])
                for q in range(CK):
                    tq = tqpool.tile([P, HALF], bf16, tag="tq", bufs=3)
                    nc.vector.tensor_scalar(
                        tq, b_bc[:, sl], lk_cols[:, q:q + 1], None, op0=ALU.is_lt
                    )
                    nc.tensor.matmul(
                        ps_gcnt[:, sl],
                        lhsT=ones_b,
                        rhs=tq,
                        start=(q == 0),
                        stop=(q == CK - 1),
                    )
                nc.vector.scalar_tensor_tensor(
                    mask_row[:, sl], ps_gcnt[:, sl], float(NKEEP), rng_row[:, sl],
                    op0=ALU.is_lt, op1=ALU.mult,
                )

            # alpha = sum(S) / sum(S * sel), folded into the mask row so
            # pass 2 is a plain 2-operand multiply. rng_row doubles as the
            # scratch output (its gate values were consumed by mask_row).
            stats = rows.tile([1, 3], f32, tag="stats")
            nc.vector.scalar_tensor_tensor(
                rng_row, ps_gcnt, float(NKEEP), s_row,
                op0=ALU.is_lt, op1=ALU.mult, accum_out=stats[:, 0:1],
            )
            nc.vector.tensor_reduce(stats[:, 1:2], s_row, axis=AXIS.X, op=ALU.add)
            nc.vector.reciprocal(stats[:, 2:3], stats[:, 0:1])
            nc.vector.tensor_scalar(
                stats[:, 2:3], stats[:, 2:3], stats[:, 1:2], None, op0=ALU.mult
            )
            maskf_row = rows.tile([1, C], bf16, tag="maskf_row")
            nc.vector.tensor_scalar(
                maskf_row, mask_row, stats[:, 2:3], None, op0=ALU.mult
            )
            for h in range(2):
                sl = slice(h * HALF, (h + 1) * HALF)
                for g in range(TWO):
                    nc.gpsimd.partition_broadcast(
                        mask_bc[:, g * C + h * HALF:g * C + (h + 1) * HALF],
                        maskf_row[:, sl],
                    )

            # ---- pass 2: out = x * maskf in bf16, SWDGE cast-store. The
            # multiplies hide under the HBM-capped store burst. The last
            # HOLD tiles' stores are deferred to the next sample's chain
            # window (released by the gate above). ----
            for t in range(nt):
                nc.vector.tensor_mul(xts[t], xts[t], mask_bc)
                if t < nt - HOLD:
                    nc.gpsimd.dma_start(out=ot[s, t], in_=xts[t])
                else:
                    held.append((s, xts, t))

        # flush the final sample's held stores
        for ps, pxts, pt in held:
            nc.gpsimd.dma_start(out=ot[ps, pt], in_=pxts[pt])


def build_nc(ns=NS, hw=HW):
    nc = bacc.Bacc(
        "TRN2", target_bir_lowering=False, debug=False, num_devices=NCORES
    )
    x = nc.dram_tensor("x", [ns, hw, C], f32, kind="ExternalInput").ap()
    r = nc.dram_tensor("r", [ns, C], f32, kind="ExternalInput").ap()
    u = nc.dram_tensor("u", [ns, C], f32, kind="ExternalInput").ap()
    o = nc.dram_tensor("o", [ns, hw, C], f32, kind="ExternalOutput").ap()
    with tile.TileContext(nc) as tc:
        emit(tc, o, x, r, u, ns, hw)
    nc.compile()
    return nc


_cached_nc = None


def kernel(x, r, u):
    global _cached_nc
    if _cached_nc is None:
        _cached_nc = build_nc()
    in_maps = [
        {
            "x": np.ascontiguousarray(x[i * NS:(i + 1) * NS], dtype=np.float32),
            "r": np.ascontiguousarray(r[i * NS:(i + 1) * NS], dtype=np.float32),
            "u": np.ascontiguousarray(u[i * NS:(i + 1) * NS], dtype=np.float32),
        }
        for i in range(NCORES)
    ]
    res = run_bass_kernel_spmd(_cached_nc, in_maps, list(range(NCORES))).results
    return np.concatenate([res[i]["o"] for i in range(NCORES)], axis=0)
